# revision 1
# baseline (speedup 1.0000x reference)
"""Trainium2 Bass kernel for nn_Big_MPNN (gnn_message_passing).

Self-contained: hardcodes shapes/sharding. Data-parallel over the batch dim
across 8 NeuronCores (16 graphs per core), weights replicated; no collectives.

Node layout trick: the host sorts nodes by GRU atom-type within each PAIR of
graphs and pads each pair to a fixed 384 columns (per-type capacities uniform
across all pairs/cores, computed from the data at runtime). g is permuted and
zero-padded on the host to match. With that, every per-type GRU matmul reads a
static strided access pattern — no gather/scatter instructions are needed on
the device, and the padded order persists across all 3 message passes.

Per-core dataflow (3 passes), transposed activations [D=128 part, cols]:
  - bond MLP layers 0..6: weights stationary f32r, nodes moving; ReLU on
    ACT/DVE alternating; layer 7 flipped (activation chunks stationary, f16)
    producing normal-layout xb for the aggregation.
  - aggregation m^T = xb^T g^T per (pair, bond) over 3 row-chunks, f16.
  - GRU per (type, branch): 6 matmuls into PSUM over pair-strided segments,
    gates on ACT (sigmoid/tanh) + DVE elementwise.
Output is DMA'd in padded-transposed form; the host unpads/unpermutes.
"""

import numpy as np

import concourse.bass as bass
import concourse.bacc as bacc
import concourse.tile as tile
import concourse.mybir as mybir

F32 = mybir.dt.float32
F32R = mybir.dt.float32r
F16 = mybir.dt.float16
AF = mybir.ActivationFunctionType
ALU = mybir.AluOpType

M = 8                      # cores
B, N, FEAT, D = 128, 128, 75, 128
NB, NL, NT = 7, 8, 6       # bonds, mlp layers, gru type slots
PASSES = 3
BG = B // M                # graphs per core
NPAIR = BG // 2            # graph pairs per core (8)
TOP_ATOMS = [6.0, 7.0, 8.0, 9.0, 0.0]

# dtype knobs
MLP_DT = F32R              # layers 0..6 activations + weights
GRU_X_DT = F32R            # Wih (x-side gru weights)
GRU_M_DT = F32R            # m path: mnT/muT + Whh
AGG_DT = F16               # x7, xb8, g'', W8


def _np_dt(dt):
    return np.float32 if dt in (F32, F32R) else np.float16


def _prepare(g, h, msg_W, gru_Wih, gru_Whh, gru_bih, gru_bhh):
    g = np.ascontiguousarray(np.asarray(g, np.float32))
    h = np.ascontiguousarray(np.asarray(h, np.float32))
    msg_W = np.asarray(msg_W, np.float32)
    gru_Wih = np.asarray(gru_Wih, np.float32).reshape(2, NT, 3, D, D)
    gru_Whh = np.asarray(gru_Whh, np.float32).reshape(2, NT, 3, D, D)
    gru_bih = np.asarray(gru_bih, np.float32).reshape(2, NT, 3, D)
    gru_bhh = np.asarray(gru_bhh, np.float32).reshape(2, NT, 3, D)

    atoms = h[:, :, 0]
    tid = np.full((B, N), NT - 1, np.int32)
    for i, a in enumerate(TOP_ATOMS):
        tid[atoms == np.float32(a)] = i
    tid_pairs = tid.reshape(B // 2, 2 * N)          # all pairs, all cores

    # per-type capacities = max count over all pairs
    counts = np.stack([(tid_pairs == t).sum(axis=1) for t in range(NT)], 1)
    caps = tuple(int(np.ceil(c / 4) * 4) for c in counts.max(axis=0))
    total = sum(caps)
    PP = int(np.ceil(total / 128) * 128)            # padded pair width
    NP = NPAIR * PP                                 # padded per-core cols
    offs = np.cumsum([0] + list(caps))[:-1]         # segment offsets in pair

    # replicated weights, partition-major layouts
    mwT = np.transpose(msg_W, (3, 0, 1, 2))         # [din, k, l, dout]
    mwT06 = np.ascontiguousarray(mwT[:, :, :NL - 1]).astype(_np_dt(MLP_DT))
    mw8T = np.ascontiguousarray(mwT[:, :, NL - 1]).astype(_np_dt(AGG_DT))
    wihT = np.ascontiguousarray(
        np.transpose(gru_Wih, (4, 0, 1, 2, 3))).astype(_np_dt(GRU_X_DT))
    whhT = np.ascontiguousarray(
        np.transpose(gru_Whh, (4, 0, 1, 2, 3))).astype(_np_dt(GRU_M_DT))
    brz = np.ascontiguousarray(
        np.transpose(gru_bih[:, :, :2] + gru_bhh[:, :, :2], (3, 0, 1, 2)))
    binn = np.ascontiguousarray(np.transpose(gru_bih[:, :, 2], (2, 0, 1)))
    bhnn = np.ascontiguousarray(np.transpose(gru_bhh[:, :, 2], (2, 0, 1)))

    h_t = np.concatenate([h, np.zeros((B, N, D - FEAT), np.float32)], axis=2)

    in_maps = []
    placements = []       # per core: padded position of each original node
    for c in range(M):
        pos = np.zeros((BG, N), np.int64)           # padded col per node
        x0 = np.zeros((NP, D), np.float32)
        gP = np.zeros((128, 3 * NPAIR, NB, PP), np.float32)  # [m-part, mchunk*pair, k, n]
        for p in range(NPAIR):
            ga, gb = c * BG + 2 * p, c * BG + 2 * p + 1
            tp = np.concatenate([tid[ga], tid[gb]])            # [256]
            hp = np.concatenate([h_t[ga], h_t[gb]], axis=0)    # [256, D]
            ppos = np.zeros(2 * N, np.int64)
            for t in range(NT):
                idx = np.flatnonzero(tp == t)
                ppos[idx] = p * PP + offs[t] + np.arange(len(idx))
            pos[2 * p] = ppos[:N]
            pos[2 * p + 1] = ppos[N:]
            x0[ppos] = hp
            # padded pair adjacency: gP[m, n] = g[graph, m_orig, n_orig]
            lp = ppos - p * PP                                  # local cols
            for gi, gr in enumerate((ga, gb)):
                li = lp[gi * N:(gi + 1) * N]
                # g[gr, k, n, m] -> block[m_pad, k, n_pad] (transposed)
                blk = np.transpose(g[gr], (2, 0, 1))            # [m, k, n]
                mc, mr = np.divmod(li, 128)
                gP[mr[:, None], (3 * p + mc)[:, None], :, li[None, :]] = \
                    np.transpose(blk, (0, 2, 1))[:, :, :]
        placements.append(pos)
        in_maps.append(dict(
            gP=gP.astype(_np_dt(AGG_DT)),
            x0=np.ascontiguousarray(x0.T).astype(_np_dt(MLP_DT)),
            mwT06=mwT06, mw8T=mw8T, wihT=wihT, whhT=whhT,
            brz=brz, binn=binn, bhnn=bhnn,
        ))
    meta = (caps, PP)
    return in_maps, meta, placements


def _build(meta):
    caps, PP = meta
    NP = NPAIR * PP
    NCH = NP // 128            # 128-col chunks (normal-layout partition chunks)
    nc = bacc.Bacc("TRN2", target_bir_lowering=False, debug=False, num_devices=M)

    gP_d = nc.dram_tensor("gP", [128, 3 * NPAIR, NB, PP], AGG_DT, kind="ExternalInput")
    x0_d = nc.dram_tensor("x0", [128, NP], MLP_DT, kind="ExternalInput")
    mwT06_d = nc.dram_tensor("mwT06", [128, NB, NL - 1, 128], MLP_DT, kind="ExternalInput")
    mw8T_d = nc.dram_tensor("mw8T", [128, NB, 128], AGG_DT, kind="ExternalInput")
    wihT_d = nc.dram_tensor("wihT", [128, 2, NT, 3, 128], GRU_X_DT, kind="ExternalInput")
    whhT_d = nc.dram_tensor("whhT", [128, 2, NT, 3, 128], GRU_M_DT, kind="ExternalInput")
    brz_d = nc.dram_tensor("brz", [128, 2, NT, 2], F32, kind="ExternalInput")
    binn_d = nc.dram_tensor("binn", [128, 2, NT], F32, kind="ExternalInput")
    bhnn_d = nc.dram_tensor("bhnn", [128, 2, NT], F32, kind="ExternalInput")
    y_d = nc.dram_tensor("y", [128, NP], F32, kind="ExternalOutput")

    # gru segment pieces: (type, col-offset-in-pair, n_pairs_start, n_cols)
    pieces = []
    off = 0
    for t in range(NT):
        if caps[t] == 0:
            off += caps[t]
            continue
        per = max(1, min(NPAIR, 512 // caps[t]))
        p0 = 0
        while p0 < NPAIR:
            npr = min(per, NPAIR - p0)
            pieces.append((t, off, p0, npr, caps[t]))
            p0 += npr
        off += caps[t]

    eng_i = [0]

    def copy_engine():
        eng_i[0] += 1
        return nc.scalar if eng_i[0] % 2 == 0 else nc.vector

    with tile.TileContext(nc) as tc:
        with (
            tc.tile_pool(name="const", bufs=1) as cp,
            tc.tile_pool(name="wmlp", bufs=2) as wp,
            tc.tile_pool(name="gpp", bufs=2) as gpp,
            tc.tile_pool(name="xp", bufs=2) as xp,
            tc.tile_pool(name="mlp", bufs=2) as mp,
            tc.tile_pool(name="x7p", bufs=1) as x7p,
            tc.tile_pool(name="xb8p", bufs=NB) as xb8p,
            tc.tile_pool(name="mtp", bufs=1) as mtp,
            tc.tile_pool(name="gates", bufs=12) as gtp,
            tc.tile_pool(name="fin", bufs=1) as fin,
            tc.tile_pool(name="ps", bufs=4, space="PSUM") as psp,
        ):
            mw8T = cp.tile([128, NB, 128], AGG_DT, tag="mw8T")
            wih = cp.tile([128, 2, NT, 3, 128], GRU_X_DT, tag="wih")
            whh = cp.tile([128, 2, NT, 3, 128], GRU_M_DT, tag="whh")
            brz = cp.tile([128, 2, NT, 2], F32, tag="brz")
            binn = cp.tile([128, 2, NT], F32, tag="binn")
            bhnn = cp.tile([128, 2, NT], F32, tag="bhnn")
            nc.sync.dma_start(mw8T[:], mw8T_d.ap())
            nc.sync.dma_start(wih[:], wihT_d.ap())
            nc.sync.dma_start(whh[:], whhT_d.ap())
            nc.sync.dma_start(brz[:], brz_d.ap())
            nc.sync.dma_start(binn[:], binn_d.ap())
            nc.sync.dma_start(bhnn[:], bhnn_d.ap())

            x_cur = xp.tile([128, NP], MLP_DT, tag="x")
            nc.sync.dma_start(x_cur[:], x0_d.ap())

            for p in range(PASSES):
                last = p == PASSES - 1
                # ---- bond MLPs ----
                xb8 = []
                for k in range(NB):
                    mw = wp.tile([128, NL - 1, 128], MLP_DT, tag="mw")
                    nc.sync.dma_start(mw[:], mwT06_d.ap()[:, k])
                    cur = x_cur
                    for l in range(NL - 1):
                        if l == NL - 2:
                            nxt = x7p.tile([128, NP], AGG_DT, tag="x7")
                        else:
                            nxt = mp.tile([128, NP], MLP_DT, tag="mlp")
                        for c2 in range(NP // 1024):
                            ps = psp.tile([128, 1024], F32, tag="ps")
                            for hh in range(2):
                                sl = slice(c2 * 1024 + hh * 512,
                                           c2 * 1024 + (hh + 1) * 512)
                                nc.tensor.matmul(ps[:, hh * 512:(hh + 1) * 512],
                                                 mw[:, l, :], cur[:, sl],
                                                 start=True, stop=True)
                            eng = copy_engine()
                            osl = slice(c2 * 1024, (c2 + 1) * 1024)
                            if eng is nc.scalar:
                                nc.scalar.activation(nxt[:, osl], ps[:], AF.Relu)
                            else:
                                nc.vector.tensor_scalar_max(nxt[:, osl], ps[:], 0.0)
                        cur = nxt
                    # layer 7 flipped -> xb normal [node-chunk part, chunk, dout]
                    xb = xb8p.tile([128, NCH, 128], AGG_DT, tag="xb8")
                    for grp in range(NCH // 4):
                        ps = psp.tile([128, 4, 128], F32, tag="ps")
                        for j in range(4):
                            ci = grp * 4 + j
                            nc.tensor.matmul(ps[:, j, :],
                                             cur[:, ci * 128:(ci + 1) * 128],
                                             mw8T[:, k, :], start=True, stop=True)
                        eng = copy_engine()
                        out_ap = xb[:, grp * 4:(grp + 1) * 4, :]
                        if eng is nc.scalar:
                            nc.scalar.copy(out_ap, ps[:])
                        else:
                            nc.vector.tensor_copy(out_ap, ps[:])
                    xb8.append(xb)

                # ---- aggregation: m^T per pair (3 m-chunks) ----
                mnT = mtp.tile([128, NP], GRU_M_DT, tag="mnT")
                muT = mtp.tile([128, NP], GRU_M_DT, tag="muT")
                for pr in range(NPAIR):
                    ps_n = psp.tile([128, PP], F32, tag="ps")
                    ps_u = psp.tile([128, PP], F32, tag="ps")
                    for mc in range(3):
                        gt = gpp.tile([128, NB, PP], AGG_DT, tag="gt")
                        nc.sync.dma_start(gt[:], gP_d.ap()[:, 3 * pr + mc])
                        for k in range(NB - 1):
                            nc.tensor.matmul(
                                ps_n[:], xb8[k][:, 3 * pr + mc, :],
                                gt[:, k, :],
                                start=(k == 0 and mc == 0),
                                stop=(k == NB - 2 and mc == 2))
                        nc.tensor.matmul(ps_u[:], xb8[NB - 1][:, 3 * pr + mc, :],
                                         gt[:, NB - 1, :],
                                         start=(mc == 0), stop=(mc == 2))
                    osl = slice(pr * PP, (pr + 1) * PP)
                    for ps_t, dst in ((ps_n, mnT), (ps_u, muT)):
                        eng = copy_engine()
                        if eng is nc.scalar:
                            nc.scalar.copy(dst[:, osl], ps_t[:])
                        else:
                            nc.vector.tensor_copy(dst[:, osl], ps_t[:])

                # ---- GRU over type segments (pair-strided APs) ----
                if last:
                    x_next = mp.tile([128, NP], F32, tag="mlp")
                else:
                    x_next = xp.tile([128, NP], MLP_DT, tag="x")
                used = sum(caps)
                if used < PP:
                    for pr in range(NPAIR):
                        nc.vector.memset(
                            x_next[:, pr * PP + used:(pr + 1) * PP].bitcast(F32),
                            0.0)

                def seg(tile_, piece):
                    t, o, p0, npr, w = piece
                    return tile_[:].rearrange("d (pr pp) -> d pr pp", pp=PP)[
                        :, p0:p0 + npr, o:o + w]

                for piece in pieces:
                    t, o, p0, npr, w = piece
                    ncols = npr * w
                    xs_ap = seg(x_cur, piece)
                    hu = []
                    for u in range(2):
                        ms_ap = seg(mnT if u == 0 else muT, piece)
                        ps_rz = psp.tile([128, 2, 512], F32, tag="ps")
                        ps_n2 = psp.tile([128, 2, 512], F32, tag="ps")
                        for gi in range(2):
                            nc.tensor.matmul(ps_rz[:, gi, :ncols], wih[:, u, t, gi, :],
                                             xs_ap, start=True, stop=False)
                            nc.tensor.matmul(ps_rz[:, gi, :ncols], whh[:, u, t, gi, :],
                                             ms_ap, start=False, stop=True)
                        nc.tensor.matmul(ps_n2[:, 0, :ncols], wih[:, u, t, 2, :],
                                         xs_ap, start=True, stop=True)
                        nc.tensor.matmul(ps_n2[:, 1, :ncols], whh[:, u, t, 2, :],
                                         ms_ap, start=True, stop=True)
                        r = gtp.tile([128, 512], F16, tag="gt")
                        z = gtp.tile([128, 512], F16, tag="gt")
                        nc.scalar.activation(r[:, :ncols], ps_rz[:, 0, :ncols],
                                             AF.Sigmoid, bias=brz[:, u, t, 0:1])
                        nc.scalar.activation(z[:, :ncols], ps_rz[:, 1, :ncols],
                                             AF.Sigmoid, bias=brz[:, u, t, 1:2])
                        t1 = gtp.tile([128, 512], F16, tag="gt")
                        nc.vector.scalar_tensor_tensor(
                            t1[:, :ncols], ps_n2[:, 1, :ncols], bhnn[:, u, t:t + 1],
                            r[:, :ncols], op0=ALU.add, op1=ALU.mult)
                        na = gtp.tile([128, 512], F16, tag="gt")
                        nc.vector.scalar_tensor_tensor(
                            na[:, :ncols], ps_n2[:, 0, :ncols], binn[:, u, t:t + 1],
                            t1[:, :ncols], op0=ALU.add, op1=ALU.add)
                        n = gtp.tile([128, 512], F16, tag="gt")
                        nc.scalar.activation(n[:, :ncols], na[:, :ncols], AF.Tanh)
                        d_ = gtp.tile([128, 512], F16, tag="gt")
                        nc.vector.tensor_sub(d_[:, :ncols], ms_ap, n[:, :ncols])
                        e = gtp.tile([128, 512], F16, tag="gt")
                        nc.vector.tensor_mul(e[:, :ncols], z[:, :ncols], d_[:, :ncols])
                        hu_t = gtp.tile([128, 512], F16, tag="gt")
                        nc.vector.tensor_add(hu_t[:, :ncols], n[:, :ncols], e[:, :ncols])
                        hu.append(hu_t)
                    nc.vector.tensor_add(seg(x_next, piece),
                                         hu[0][:, :ncols], hu[1][:, :ncols])
                x_cur = x_next

            nc.sync.dma_start(y_d.ap(), x_cur[:])

    nc.compile()
    return nc


def _make_runner(nc):
    import jax
    from jax.experimental.shard_map import shard_map
    from jax.sharding import Mesh, PartitionSpec, NamedSharding
    from concourse.bass2jax import (install_neuronx_cc_hook, _bass_exec_p,
                                    partition_id_tensor)

    install_neuronx_cc_hook()
    partition_name = (nc.partition_id_tensor.name
                      if nc.partition_id_tensor else None)
    in_names, out_names, out_avals, zero_outs = [], [], [], []
    for alloc in nc.m.functions[0].allocations:
        if not isinstance(alloc, mybir.MemoryLocationSet):
            continue
        name = alloc.memorylocations[0].name
        if alloc.kind == "ExternalInput":
            if name != partition_name:
                in_names.append(name)
        elif alloc.kind == "ExternalOutput":
            out_names.append(name)
            shape = tuple(alloc.tensor_shape)
            dtype = mybir.dt.np(alloc.dtype)
            out_avals.append(jax.core.ShapedArray(shape, dtype))
            zero_outs.append(np.zeros(shape, dtype))
    n_params = len(in_names)
    all_names = in_names + out_names
    if partition_name is not None:
        all_names = all_names + [partition_name]

    def _body(*args):
        operands = list(args)
        if partition_name is not None:
            operands.append(partition_id_tensor())
        outs = _bass_exec_p.bind(
            *operands,
            out_avals=tuple(out_avals),
            in_names=tuple(all_names),
            out_names=tuple(out_names),
            lowering_input_output_aliases=(),
            sim_require_finite=True,
            sim_require_nnan=True,
            nc=nc,
        )
        return tuple(outs)

    devices = jax.devices()[:M]
    mesh = Mesh(np.asarray(devices), ("core",))
    specs = (PartitionSpec("core"),) * (n_params + len(out_names))
    fn = jax.jit(shard_map(_body, mesh=mesh,
                           in_specs=specs,
                           out_specs=(PartitionSpec("core"),) * len(out_names)),
                 keep_unused=True)

    def put(in_maps):
        sh = NamedSharding(mesh, PartitionSpec("core"))
        args = []
        for name in in_names:
            cat = np.concatenate([np.asarray(im[name]) for im in in_maps], axis=0)
            args.append(jax.device_put(cat, sh))
        for z in zero_outs:
            cat = np.concatenate([z] * M, axis=0)
            args.append(jax.device_put(cat, sh))
        return args

    def run(args):
        outs = fn(*args)
        outs = [np.asarray(o) for o in outs]
        per_core = []
        for c in range(M):
            per_core.append({
                name: outs[i].reshape(M, *out_avals[i].shape)[c]
                for i, name in enumerate(out_names)})
        return per_core

    return put, run


_CACHE = {}


def _get_runner(meta):
    if meta not in _CACHE:
        nc = _build(meta)
        _CACHE[meta] = (_make_runner(nc), nc)
    return _CACHE[meta]


def _assemble(per_core, placements):
    out = np.empty((B, N, D), np.float32)
    for c in range(M):
        y = per_core[c]["y"]                      # [D, NP] padded transposed
        pos = placements[c]                       # [BG, N]
        out[c * BG:(c + 1) * BG] = y.T[pos]       # gather real columns
    return out


def kernel(g, h, msg_W, gru_Wih, gru_Whh, gru_bih, gru_bhh):
    in_maps, meta, placements = _prepare(g, h, msg_W, gru_Wih, gru_Whh,
                                         gru_bih, gru_bhh)
    (put, run), _nc = _get_runner(meta)
    args = put(in_maps)
    per_core = run(args)
    return _assemble(per_core, placements)


# exposed for test.py
def get_nc_and_runner(g, h, msg_W, gru_Wih, gru_Whh, gru_bih, gru_bhh):
    in_maps, meta, placements = _prepare(g, h, msg_W, gru_Wih, gru_Whh,
                                         gru_bih, gru_bhh)
    (put, run), nc = _get_runner(meta)
    return in_maps, put, run, nc, placements



# revision 8
# speedup vs baseline: 1.4201x; 1.4201x over previous
"""Trainium2 Bass kernel for nn_Big_MPNN (gnn_message_passing).

Self-contained: hardcodes shapes/sharding. Data-parallel over the batch dim
across 8 NeuronCores (16 graphs per core), weights replicated; no collectives.

Layout ("compact-m"): the per-node state x lives in COMPACT original graph
order (2048 cols per core, transposed [D=128 part, 2048]) — the bond MLP runs
with zero padding waste. The per-atom-type sort needed by the grouped GRU is
folded into the aggregation matmul: the host permutes/pads g's TARGET columns
into a per-pair type-sorted space of width PP (caps uniform across pairs), and
appends a one-hot permutation block P as an 8th "bond" slot, so the
aggregation emits both messages m and the sorted state xs directly in sorted
space. After the GRU, a one-hot inverse permutation PT (28 small matmuls)
returns h' to compact order for the next pass. All matmul contractions per
pair touch exactly its 2 original 128-chunks.

Engines: f16 datapath with f32 PSUM accumulation; ReLU/PSUM-copies split
across ACT+DVE by a static cost balancer (GPSIMD/DMA cannot touch PSUM);
SBUF-side f16 GRU elementwise goes to DVE (2x mode) or Pool; per-pair g
tiles stream from HBM with an 8-deep prefetch ring.
"""

import numpy as np

import concourse.bass as bass
import concourse.bacc as bacc
import concourse.tile as tile
import concourse.mybir as mybir

F32 = mybir.dt.float32
F16 = mybir.dt.float16
AF = mybir.ActivationFunctionType
ALU = mybir.AluOpType

M = 8                      # cores
B, N, FEAT, D = 128, 128, 75, 128
NB, NL, NT = 7, 8, 6       # bonds, mlp layers, gru type slots
PASSES = 3
BG = B // M                # graphs per core (16)
NPAIR = BG // 2            # graph pairs per core (8)
NCO = BG * N               # compact cols per core (2048)
TOP_ATOMS = [6.0, 7.0, 8.0, 9.0, 0.0]


def _prepare(g, h, msg_W, gru_Wih, gru_Whh, gru_bih, gru_bhh):
    g = np.ascontiguousarray(np.asarray(g, np.float32))
    h = np.ascontiguousarray(np.asarray(h, np.float32))
    msg_W = np.asarray(msg_W, np.float32)
    gru_Wih = np.asarray(gru_Wih, np.float32).reshape(2, NT, 3, D, D)
    gru_Whh = np.asarray(gru_Whh, np.float32).reshape(2, NT, 3, D, D)
    gru_bih = np.asarray(gru_bih, np.float32).reshape(2, NT, 3, D)
    gru_bhh = np.asarray(gru_bhh, np.float32).reshape(2, NT, 3, D)

    atoms = h[:, :, 0]
    tid = np.full((B, N), NT - 1, np.int32)
    for i, a in enumerate(TOP_ATOMS):
        tid[atoms == np.float32(a)] = i
    tid_pairs = tid.reshape(B // 2, 2 * N)          # all pairs, all cores

    # per-type capacities = max count over all pairs, rounded to 4
    counts = np.stack([(tid_pairs == t).sum(axis=1) for t in range(NT)], 1)
    caps = tuple(int(np.ceil(c / 4) * 4) for c in counts.max(axis=0))
    used = sum(caps)
    PP = int(np.ceil(used / 16) * 16)               # sorted width per pair
    NS = NPAIR * PP                                 # sorted cols per core
    offs = np.cumsum([0] + list(caps))[:-1]

    # replicated weights, partition-major layouts, f16
    mwT06 = np.ascontiguousarray(
        np.transpose(msg_W, (3, 0, 1, 2))[:, :, :NL - 1]).astype(np.float16)
    mw8T = np.ascontiguousarray(
        np.transpose(msg_W, (3, 0, 1, 2))[:, :, NL - 1]).astype(np.float16)
    wihT = np.ascontiguousarray(
        np.transpose(gru_Wih, (4, 0, 1, 2, 3))).astype(np.float16)
    whhT = np.ascontiguousarray(
        np.transpose(gru_Whh, (4, 0, 1, 2, 3))).astype(np.float16)
    brz = np.ascontiguousarray(
        np.transpose(gru_bih[:, :, :2] + gru_bhh[:, :, :2], (3, 0, 1, 2)))
    binn = np.ascontiguousarray(np.transpose(gru_bih[:, :, 2], (2, 0, 1)))
    bhnn = np.ascontiguousarray(np.transpose(gru_bhh[:, :, 2], (2, 0, 1)))
    ident = np.eye(D, dtype=np.float16)

    h_t = np.concatenate([h, np.zeros((B, N, D - FEAT), np.float32)], axis=2)

    # per-pair sorted-chunk lists (global 128-grid over the sorted space)
    cranges = []
    for pr in range(NPAIR):
        lo = (pr * PP) // 128
        hi = ((pr + 1) * PP - 1) // 128
        cranges.append(list(range(lo, hi + 1)))
    nslots = sum(len(c) for c in cranges)

    in_maps = []
    placements = []       # per core: sorted position of each original node
    for c in range(M):
        x0 = np.ascontiguousarray(
            h_t[c * BG:(c + 1) * BG].reshape(NCO, D).T).astype(np.float16)
        gP = np.zeros((128, BG, NB + 1, PP), np.float32)   # [m_r, chunk, slot, ns]
        PT = np.zeros((128, nslots, 256), np.float32)      # [ns_r, slot, m_local]
        pos = np.zeros((BG, N), np.int64)
        slot0 = 0
        for pr in range(NPAIR):
            ga, gb = c * BG + 2 * pr, c * BG + 2 * pr + 1
            tp = np.concatenate([tid[ga], tid[gb]])            # [256]
            spos = np.zeros(2 * N, np.int64)
            for t in range(NT):
                idx = np.flatnonzero(tp == t)
                spos[idx] = offs[t] + np.arange(len(idx))
            pos[2 * pr] = pr * PP + spos[:N]
            pos[2 * pr + 1] = pr * PP + spos[N:]
            for gi, gr in enumerate((ga, gb)):
                ch = 2 * pr + gi
                sp = spos[gi * N:(gi + 1) * N]                 # [128] in-pair
                # bonds: gP[j, ch, k, sp[n]] = g[gr, k, n, j]
                # (scalar ch + array sp => advanced dims move to front: (n,j,k))
                gP[:, ch, :NB, sp] = np.transpose(g[gr], (1, 2, 0))
                # P: one-hot own sorted position
                gP[np.arange(N), ch, NB, sp] = 1.0
            # PT: inverse permutation, per touched global sorted chunk
            for si, cg in enumerate(cranges[pr]):
                s = slot0 + si
                gc = pr * PP + spos                            # global col
                ml = np.flatnonzero((gc >= cg * 128) & (gc < (cg + 1) * 128))
                PT[gc[ml] - cg * 128, s, ml] = 1.0
            slot0 += len(cranges[pr])
        placements.append(pos)
        in_maps.append(dict(
            gP=gP.astype(np.float16), PT=PT.astype(np.float16),
            x0=x0, mwT06=mwT06, mw8T=mw8T, wihT=wihT, whhT=whhT,
            brz=brz, binn=binn, bhnn=bhnn, ident=ident,
        ))
    meta = (caps, PP)
    return in_maps, meta, placements


class _Balancer:
    """Static greedy assignment of elementwise ops to ACT/DVE/Pool by
    estimated cost-model ns."""

    def __init__(self, nc):
        self.nc = nc
        self.load = {"act": 0.0, "dve": 0.0, "pool": 0.0}

    @staticmethod
    def _cost(eng, cols, kind):
        if eng == "act":
            return cols * 0.8333 + 242.0
        if eng == "dve":
            if kind == "psum32":
                return cols * 1.0417 + 195.0
            if kind == "psum16":
                return cols * 0.5208 + 195.0
            return cols * (0.5208 if kind == "tt16" else 0.2604) + 130.0
        # pool (sbuf-only)
        return cols * (1.9841 if kind == "tt16" else 1.3889) + 156.0

    def _pick(self, engines, cols, kind):
        best, bc = None, None
        for e in engines:
            c = self._cost(e, cols, kind)
            if best is None or self.load[e] + c < bc:
                best, bc = e, self.load[e] + c
        self.load[best] += self._cost(best, cols, kind)
        return best

    def charge(self, eng, cols, kind="psum32"):
        self.load[eng] += self._cost(eng, cols, kind)

    def relu(self, out, in_, cols):
        e = self._pick(("act", "dve"), cols, "psum32")
        if e == "act":
            self.nc.scalar.activation(out, in_, AF.Relu)
        else:
            self.nc.vector.tensor_scalar_max(out, in_, 0.0)

    def copy(self, out, in_, cols, kind="psum32"):
        e = self._pick(("act", "dve"), cols, kind)
        if e == "act":
            self.nc.scalar.copy(out, in_)
        else:
            self.nc.vector.tensor_copy(out, in_)

    def tt(self, op, out, a, b, cols):
        e = self._pick(("dve", "pool"), cols, "tt16")
        eng = self.nc.vector if e == "dve" else self.nc.gpsimd
        getattr(eng, op)(out, a, b)


def _build(meta):
    caps, PP = meta
    used = sum(caps)
    NS = NPAIR * PP
    NCHS = NS // 128
    offs = np.cumsum([0] + list(caps))[:-1]
    nc = bacc.Bacc("TRN2", target_bir_lowering=False, debug=False, num_devices=M)

    cranges = []
    for pr in range(NPAIR):
        lo = (pr * PP) // 128
        hi = ((pr + 1) * PP - 1) // 128
        cranges.append(list(range(lo, hi + 1)))
    nslots = sum(len(c) for c in cranges)

    gP_d = nc.dram_tensor("gP", [128, BG, NB + 1, PP], F16, kind="ExternalInput")
    PT_d = nc.dram_tensor("PT", [128, nslots, 256], F16, kind="ExternalInput")
    x0_d = nc.dram_tensor("x0", [128, NCO], F16, kind="ExternalInput")
    mwT06_d = nc.dram_tensor("mwT06", [128, NB, NL - 1, 128], F16, kind="ExternalInput")
    mw8T_d = nc.dram_tensor("mw8T", [128, NB, 128], F16, kind="ExternalInput")
    wihT_d = nc.dram_tensor("wihT", [128, 2, NT, 3, 128], F16, kind="ExternalInput")
    whhT_d = nc.dram_tensor("whhT", [128, 2, NT, 3, 128], F16, kind="ExternalInput")
    brz_d = nc.dram_tensor("brz", [128, 2, NT, 2], F32, kind="ExternalInput")
    binn_d = nc.dram_tensor("binn", [128, 2, NT], F32, kind="ExternalInput")
    bhnn_d = nc.dram_tensor("bhnn", [128, 2, NT], F32, kind="ExternalInput")
    ident_d = nc.dram_tensor("ident", [128, 128], F16, kind="ExternalInput")
    y_d = nc.dram_tensor("y", [128, NS], F16, kind="ExternalOutput")

    # gru pieces: (type, col-offset-in-pair, pair_start, n_pairs, width)
    pieces = []
    for t in range(NT):
        if caps[t] == 0:
            continue
        per = max(1, min(NPAIR, 512 // caps[t]))
        p0 = 0
        while p0 < NPAIR:
            npr = min(per, NPAIR - p0)
            pieces.append((t, int(offs[t]), p0, npr, caps[t]))
            p0 += npr

    with tile.TileContext(nc) as tc:
        with (
            tc.tile_pool(name="const", bufs=1) as cp,
            tc.tile_pool(name="xp", bufs=2) as xp,
            tc.tile_pool(name="mlp", bufs=4) as mp,
            tc.tile_pool(name="x7p", bufs=2) as x7p,
            tc.tile_pool(name="xb8p", bufs=NB) as xb8p,
            tc.tile_pool(name="xnp", bufs=1) as xnp,
            tc.tile_pool(name="gtp", bufs=8) as gtp,
            tc.tile_pool(name="msp", bufs=1) as msp,
            tc.tile_pool(name="hsp", bufs=2) as hsp,
            tc.tile_pool(name="hsnp", bufs=1) as hsnp,
            tc.tile_pool(name="gates", bufs=12) as gp_,
            tc.tile_pool(name="psA", bufs=3, space="PSUM") as psA,
            tc.tile_pool(name="psB", bufs=2, space="PSUM") as psB,
        ):
            bal = _Balancer(nc)

            mwT = cp.tile([128, NB, NL - 1, 128], F16, tag="mwT")
            mw8T = cp.tile([128, NB, 128], F16, tag="mw8T")
            wih = cp.tile([128, 2, NT, 3, 128], F16, tag="wih")
            whh = cp.tile([128, 2, NT, 3, 128], F16, tag="whh")
            brz = cp.tile([128, 2, NT, 2], F32, tag="brz")
            binn = cp.tile([128, 2, NT], F32, tag="binn")
            bhnn = cp.tile([128, 2, NT], F32, tag="bhnn")
            ident = cp.tile([128, 128], F16, tag="ident")
            PT = cp.tile([128, nslots, 256], F16, tag="PT")
            nc.sync.dma_start(mwT[:], mwT06_d.ap())
            nc.sync.dma_start(mw8T[:], mw8T_d.ap())
            nc.sync.dma_start(wih[:], wihT_d.ap())
            nc.sync.dma_start(whh[:], whhT_d.ap())
            nc.sync.dma_start(brz[:], brz_d.ap())
            nc.sync.dma_start(binn[:], binn_d.ap())
            nc.sync.dma_start(bhnn[:], bhnn_d.ap())
            nc.sync.dma_start(ident[:], ident_d.ap())
            nc.sync.dma_start(PT[:], PT_d.ap())

            x_cur = xp.tile([128, NCO], F16, tag="x")
            nc.sync.dma_start(x_cur[:], x0_d.ap())

            def seg(t_ap, piece):
                t, o, p0, npr, w = piece
                return t_ap.rearrange("d (pr pp) -> d pr pp", pp=PP)[
                    :, p0:p0 + npr, o:o + w]

            for p in range(PASSES):
                last = p == PASSES - 1
                # ---- queue g tiles for this pass (prefetch ring) ----
                gts = []
                for ch in range(BG):
                    gt = gtp.tile([128, NB + 1, PP], F16, tag="gt")
                    nc.sync.dma_start(gt[:], gP_d.ap()[:, ch])
                    gts.append(gt)

                # ---- x-flip: xnorm[m-chunk, d] = x_cur chunks transposed ----
                xnorm = xnp.tile([128, BG, 128], F16, tag="xn")
                for grp in range(2):
                    ps = psA.tile([128, 8, 128], F16, tag="ps")
                    for j in range(8):
                        ci = grp * 8 + j
                        nc.tensor.transpose(
                            ps[:, j, :], x_cur[:, ci * 128:(ci + 1) * 128],
                            ident[:])
                    bal.copy(xnorm[:, grp * 8:(grp + 1) * 8, :], ps[:], 1024,
                             kind="psum16")

                # ---- bond MLPs (bonds interleaved in pairs) ----
                xb8 = [None] * NB
                for kgrp in ((0, 1), (2, 3), (4, 5), (6,)):
                    curs = {k: x_cur for k in kgrp}
                    for l in range(NL - 1):
                        for k in kgrp:
                            if l == NL - 2:
                                nxt = x7p.tile([128, NCO], F16, tag="x7")
                            else:
                                nxt = mp.tile([128, NCO], F16, tag="mlp")
                            for u2 in range(2):
                                ps = psA.tile([128, 1024], F32, tag="ps")
                                for hh in range(2):
                                    sl = slice(u2 * 1024 + hh * 512,
                                               u2 * 1024 + (hh + 1) * 512)
                                    nc.tensor.matmul(
                                        ps[:, hh * 512:(hh + 1) * 512],
                                        mwT[:, k, l, :], curs[k][:, sl],
                                        start=True, stop=True)
                                bal.relu(nxt[:, u2 * 1024:(u2 + 1) * 1024],
                                         ps[:], 1024)
                            curs[k] = nxt
                    # layer 7 flipped -> xb normal [m-chunk part, chunk, dout]
                    for k in kgrp:
                        xb = xb8p.tile([128, BG, 128], F16, tag="xb8")
                        for grp in range(2):
                            ps = psA.tile([128, 8, 128], F32, tag="ps")
                            for j in range(8):
                                ci = grp * 8 + j
                                nc.tensor.matmul(
                                    ps[:, j, :],
                                    curs[k][:, ci * 128:(ci + 1) * 128],
                                    mw8T[:, k, :], start=True, stop=True)
                            bal.copy(xb[:, grp * 8:(grp + 1) * 8, :], ps[:],
                                     1024)
                        xb8[k] = xb

                # ---- aggregation + P-permute, per pair ----
                m2 = msp.tile([128, 2, NS], F16, tag="m2")
                xs = msp.tile([128, NS], F16, tag="xs")
                for pr in range(NPAIR):
                    ps2 = psA.tile([128, 2, 512], F32, tag="ps")
                    psx = psB.tile([128, 512], F32, tag="psb")
                    for mc in range(2):
                        ch = 2 * pr + mc
                        gt = gts[ch]
                        for k in range(NB - 1):
                            nc.tensor.matmul(
                                ps2[:, 0, :PP], xb8[k][:, ch, :], gt[:, k, :],
                                start=(mc == 0 and k == 0),
                                stop=(mc == 1 and k == NB - 2))
                        nc.tensor.matmul(
                            ps2[:, 1, :PP], xb8[NB - 1][:, ch, :],
                            gt[:, NB - 1, :],
                            start=(mc == 0), stop=(mc == 1))
                        nc.tensor.matmul(
                            psx[:, :PP], xnorm[:, ch, :], gt[:, NB, :],
                            start=(mc == 0), stop=(mc == 1))
                    osl = slice(pr * PP, (pr + 1) * PP)
                    bal.copy(m2[:, :, osl], ps2[:, :, :PP], 2 * PP)
                    bal.copy(xs[:, osl], psx[:, :PP], PP)

                # ---- GRU over type segments (sorted space) ----
                hs = hsp.tile([128, NS], F16, tag="hs")
                if p == 0 and used < PP:
                    for bb in range(2):
                        # zero the per-pair tail holes once per buffer
                        hz = hsp.tile([128, NS], F16, tag="hs") if bb else hs
                        nc.gpsimd.memset(
                            hz[:].rearrange("d (pr pp) -> d pr pp", pp=PP)[
                                :, :, used:PP], 0.0)
                for piece in pieces:
                    t, o, p0, npr, w = piece
                    ncols = npr * w
                    xs_ap = seg(xs[:], piece)
                    hu = []
                    for u in range(2):
                        ms_ap = seg(m2[:, u], piece)
                        ps_rz = psA.tile([128, 2, 512], F32, tag="ps")
                        ps_n2 = psA.tile([128, 2, 512], F32, tag="ps")
                        for gi in range(2):
                            nc.tensor.matmul(ps_rz[:, gi, :ncols],
                                             wih[:, u, t, gi, :], xs_ap,
                                             start=True, stop=False)
                            nc.tensor.matmul(ps_rz[:, gi, :ncols],
                                             whh[:, u, t, gi, :], ms_ap,
                                             start=False, stop=True)
                        nc.tensor.matmul(ps_n2[:, 0, :ncols],
                                         wih[:, u, t, 2, :], xs_ap,
                                         start=True, stop=True)
                        nc.tensor.matmul(ps_n2[:, 1, :ncols],
                                         whh[:, u, t, 2, :], ms_ap,
                                         start=True, stop=True)
                        r = gp_.tile([128, 512], F16, tag="gt")
                        z = gp_.tile([128, 512], F16, tag="gt")
                        nc.scalar.activation(r[:, :ncols], ps_rz[:, 0, :ncols],
                                             AF.Sigmoid, bias=brz[:, u, t, 0:1])
                        nc.scalar.activation(z[:, :ncols], ps_rz[:, 1, :ncols],
                                             AF.Sigmoid, bias=brz[:, u, t, 1:2])
                        bal.charge("act", 2 * ncols)
                        t1 = gp_.tile([128, 512], F16, tag="gt")
                        nc.vector.scalar_tensor_tensor(
                            t1[:, :ncols], ps_n2[:, 1, :ncols],
                            bhnn[:, u, t:t + 1], r[:, :ncols],
                            op0=ALU.add, op1=ALU.mult)
                        na = gp_.tile([128, 512], F16, tag="gt")
                        nc.vector.scalar_tensor_tensor(
                            na[:, :ncols], ps_n2[:, 0, :ncols],
                            binn[:, u, t:t + 1], t1[:, :ncols],
                            op0=ALU.add, op1=ALU.add)
                        bal.charge("dve", 2 * ncols, kind="psum32")
                        n = gp_.tile([128, 512], F16, tag="gt")
                        nc.scalar.activation(n[:, :ncols], na[:, :ncols],
                                             AF.Tanh)
                        bal.charge("act", ncols)
                        d_ = gp_.tile([128, 512], F16, tag="gt")
                        bal.tt("tensor_sub", d_[:, :ncols], ms_ap,
                               n[:, :ncols], ncols)
                        e = gp_.tile([128, 512], F16, tag="gt")
                        bal.tt("tensor_mul", e[:, :ncols], z[:, :ncols],
                               d_[:, :ncols], ncols)
                        hu_t = gp_.tile([128, 512], F16, tag="gt")
                        bal.tt("tensor_add", hu_t[:, :ncols], n[:, :ncols],
                               e[:, :ncols], ncols)
                        hu.append(hu_t)
                    bal.tt("tensor_add", seg(hs[:], piece),
                           hu[0][:, :ncols], hu[1][:, :ncols], ncols)

                if last:
                    nc.sync.dma_start(y_d.ap(), hs[:])
                else:
                    # ---- back-permute: hs -> compact x_next ----
                    hsn = hsnp.tile([128, NCHS, 128], F16, tag="hsn")
                    c0 = 0
                    while c0 < NCHS:
                        cw = min(8, NCHS - c0)
                        ps = psA.tile([128, 8, 128], F16, tag="ps")
                        for j in range(cw):
                            nc.tensor.transpose(
                                ps[:, j, :],
                                hs[:, (c0 + j) * 128:(c0 + j + 1) * 128],
                                ident[:])
                        bal.copy(hsn[:, c0:c0 + cw, :], ps[:, :cw, :],
                                 cw * 128, kind="psum16")
                        c0 += cw
                    x_next = xp.tile([128, NCO], F16, tag="x")
                    slot = 0
                    for pr in range(NPAIR):
                        psb = psB.tile([128, 512], F32, tag="psb")
                        crs = cranges[pr]
                        for si, cg in enumerate(crs):
                            nc.tensor.matmul(
                                psb[:, :256], hsn[:, cg, :], PT[:, slot, :],
                                start=(si == 0), stop=(si == len(crs) - 1))
                            slot += 1
                        bal.copy(x_next[:, pr * 256:(pr + 1) * 256],
                                 psb[:, :256], 256)
                    x_cur = x_next

    nc.compile()
    return nc


def _make_runner(nc):
    import jax
    from jax.experimental.shard_map import shard_map
    from jax.sharding import Mesh, PartitionSpec, NamedSharding
    from concourse.bass2jax import (install_neuronx_cc_hook, _bass_exec_p,
                                    partition_id_tensor)

    install_neuronx_cc_hook()
    partition_name = (nc.partition_id_tensor.name
                      if nc.partition_id_tensor else None)
    in_names, out_names, out_avals, zero_outs = [], [], [], []
    for alloc in nc.m.functions[0].allocations:
        if not isinstance(alloc, mybir.MemoryLocationSet):
            continue
        name = alloc.memorylocations[0].name
        if alloc.kind == "ExternalInput":
            if name != partition_name:
                in_names.append(name)
        elif alloc.kind == "ExternalOutput":
            out_names.append(name)
            shape = tuple(alloc.tensor_shape)
            dtype = mybir.dt.np(alloc.dtype)
            out_avals.append(jax.core.ShapedArray(shape, dtype))
            zero_outs.append(np.zeros(shape, dtype))
    n_params = len(in_names)
    all_names = in_names + out_names
    if partition_name is not None:
        all_names = all_names + [partition_name]

    def _body(*args):
        operands = list(args)
        if partition_name is not None:
            operands.append(partition_id_tensor())
        outs = _bass_exec_p.bind(
            *operands,
            out_avals=tuple(out_avals),
            in_names=tuple(all_names),
            out_names=tuple(out_names),
            lowering_input_output_aliases=(),
            sim_require_finite=True,
            sim_require_nnan=True,
            nc=nc,
        )
        return tuple(outs)

    devices = jax.devices()[:M]
    mesh = Mesh(np.asarray(devices), ("core",))
    specs = (PartitionSpec("core"),) * (n_params + len(out_names))
    fn = jax.jit(shard_map(_body, mesh=mesh,
                           in_specs=specs,
                           out_specs=(PartitionSpec("core"),) * len(out_names)),
                 keep_unused=True)

    def put(in_maps):
        sh = NamedSharding(mesh, PartitionSpec("core"))
        args = []
        for name in in_names:
            cat = np.concatenate([np.asarray(im[name]) for im in in_maps], axis=0)
            args.append(jax.device_put(cat, sh))
        for z in zero_outs:
            cat = np.concatenate([z] * M, axis=0)
            args.append(jax.device_put(cat, sh))
        return args

    def run(args):
        outs = fn(*args)
        outs = [np.asarray(o) for o in outs]
        per_core = []
        for c in range(M):
            per_core.append({
                name: outs[i].reshape(M, *out_avals[i].shape)[c]
                for i, name in enumerate(out_names)})
        return per_core

    return put, run


_CACHE = {}


def _get_runner(meta):
    if meta not in _CACHE:
        nc = _build(meta)
        _CACHE[meta] = (_make_runner(nc), nc)
    return _CACHE[meta]


def _assemble(per_core, placements):
    out = np.empty((B, N, D), np.float32)
    for c in range(M):
        y = per_core[c]["y"].astype(np.float32)   # [D, NS] sorted transposed
        pos = placements[c]                       # [BG, N]
        out[c * BG:(c + 1) * BG] = y.T[pos]       # gather real columns
    return out


def kernel(g, h, msg_W, gru_Wih, gru_Whh, gru_bih, gru_bhh):
    in_maps, meta, placements = _prepare(g, h, msg_W, gru_Wih, gru_Whh,
                                         gru_bih, gru_bhh)
    (put, run), _nc = _get_runner(meta)
    args = put(in_maps)
    per_core = run(args)
    return _assemble(per_core, placements)


# exposed for test.py
def get_nc_and_runner(g, h, msg_W, gru_Wih, gru_Whh, gru_bih, gru_bhh):
    in_maps, meta, placements = _prepare(g, h, msg_W, gru_Wih, gru_Whh,
                                         gru_bih, gru_bhh)
    (put, run), nc = _get_runner(meta)
    return in_maps, put, run, nc, placements
